# revision 1
# baseline (speedup 1.0000x reference)
import os
import sys

for _p in ("/opt/trn_rl_repo", os.path.expanduser("~/.axon_site/_ro/trn_rl_repo")):
    if os.path.isdir(_p) and _p not in sys.path:
        sys.path.insert(0, _p)

import numpy as np
import ml_dtypes

import concourse.bass as bass
from concourse import bacc
import concourse.tile as tile
import concourse.mybir as mybir
from concourse.bass_utils import run_bass_kernel_spmd

# Problem shape (hardcoded per contract)
B, T, D, H, DK = 4, 2048, 1024, 16, 64
NCORES = 8

# Sharding: core = (batch b, head-group hg). Each core handles 8 heads of one
# batch over the full sequence, row-shards W_o, and the host sums the two
# partial outputs per batch (the "all-reduce" of the tensor-parallel scheme).
HC = H // 2       # 8 heads per core
DC = HC * DK      # 512 hidden dims per core

P = 128
NG = D // P       # 8 contraction tiles for the projections
NPAIR = HC // 2   # 4 Q/K projection tiles (2 heads each)
NKT = T // P      # 16 key-token tiles
QCH = 512         # query-chunk width
NQC = T // QCH    # 4 query chunks
NQB = QCH // P    # 4 query blocks of 128 per chunk
NGR = NKT // 2    # 8 score groups (2 key tiles each) per (chunk, head)

bf16 = mybir.dt.bfloat16
fp8 = mybir.dt.float8e4
f16 = mybir.dt.float16
f32 = mybir.dt.float32
i16 = mybir.dt.int16
FT = mybir.ActivationFunctionType
ADD = mybir.AluOpType.add
MUL = mybir.AluOpType.mult
DR = mybir.MatmulPerfMode.DoubleRow

LOG2E = 1.4426950408889634
# DVE fast-exp (int16 bitcast to bf16): i16 = round(s*ALPHA + BETA)
ALPHA = 0.125 * LOG2E * 128.0
BETA = 16256.0 - 0.5

# which of the 8 score groups per (c,h) use the DVE bit-trick exp
DVE_GROUPS = (2, 4, 6)
# consume (attnV) emission trails the produce stream by this many steps
LAGU = 6

_CACHE = {}


def build_kernel():
    nc = bacc.Bacc("TRN2", target_bir_lowering=False, debug=False, num_devices=1)

    xT = nc.dram_tensor("xT", [NG, P, T], bf16, kind="ExternalInput")
    Wq = nc.dram_tensor("Wq", [P, NG, NPAIR, P], bf16, kind="ExternalInput")
    Wk = nc.dram_tensor("Wk", [P, NG, NPAIR, P], bf16, kind="ExternalInput")
    Wv = nc.dram_tensor("Wv", [P, NG, DC], bf16, kind="ExternalInput")
    Wo = nc.dram_tensor("Wo", [P, 4, D], bf16, kind="ExternalInput")
    bqp = nc.dram_tensor("bqp", [P, NPAIR], f32, kind="ExternalInput")
    bkp = nc.dram_tensor("bkp", [P, NPAIR], f32, kind="ExternalInput")
    bv = nc.dram_tensor("bv", [1, DC], f32, kind="ExternalInput")
    bo = nc.dram_tensor("bo", [1, D], f32, kind="ExternalInput")  # pre-halved
    iden = nc.dram_tensor("iden", [P, P], bf16, kind="ExternalInput")
    out = nc.dram_tensor("out", [T, D], f16, kind="ExternalOutput")

    with tile.TileContext(nc) as tc:
        with (
            tc.tile_pool(name="big", bufs=1) as big,
            tc.tile_pool(name="pt", bufs=20) as ptp,
            tc.tile_pool(name="stg", bufs=4) as stgp,
            tc.tile_pool(name="rc", bufs=4) as rcp,
            tc.tile_pool(name="res", bufs=4) as resp,
            tc.tile_pool(name="sg", bufs=2, space="PSUM") as sgp,
            tc.tile_pool(name="av", bufs=2, space="PSUM") as avp,
            tc.tile_pool(name="acc", bufs=2, space="PSUM") as accp,
        ):
            # ---------- prefetch ----------
            wk_sb = big.tile([P, NG, NPAIR, P], bf16, name="wk_sb")
            nc.sync.dma_start(wk_sb[:], Wk[:])
            # x^T arrives in query-chunk column slices so the first K-proj
            # tiles can start ~3us in instead of waiting the full 8MB.
            xt_sb = [big.tile([P, T], bf16, name=f"xt{g}") for g in range(NG)]
            for g in range(NG):
                nc.sync.dma_start(xt_sb[g][:, 0:QCH], xT[g][:, 0:QCH])
            bk_sb = big.tile([P, NPAIR], f32, name="bk_sb")
            bq_sb = big.tile([P, NPAIR], f32, name="bq_sb")
            nc.sync.dma_start(bk_sb[:], bkp[:])
            nc.sync.dma_start(bq_sb[:], bqp[:])
            for g in range(NG):
                nc.sync.dma_start(xt_sb[g][:, QCH : 2 * QCH], xT[g][:, QCH : 2 * QCH])
            wq_sb = big.tile([P, NG, NPAIR, P], bf16, name="wq_sb")
            nc.sync.dma_start(wq_sb[:], Wq[:])
            iden_sb = big.tile([P, P], bf16, name="iden_sb")
            nc.sync.dma_start(iden_sb[:], iden[:])
            for g in range(NG):
                nc.sync.dma_start(
                    xt_sb[g][:, 2 * QCH : 3 * QCH], xT[g][:, 2 * QCH : 3 * QCH]
                )
            wv_sb = big.tile([P, NG, DC], bf16, name="wv_sb")
            nc.sync.dma_start(wv_sb[:], Wv[:])
            bv_rep = big.tile([P, DC], f32, name="bv_rep")
            nc.sync.dma_start(bv_rep[:], bv[:].to_broadcast((P, DC)))
            for g in range(NG):
                nc.sync.dma_start(
                    xt_sb[g][:, 3 * QCH : 4 * QCH], xT[g][:, 3 * QCH : 4 * QCH]
                )
            wo_sb = big.tile([P, 4, D], bf16, name="wo_sb")
            nc.sync.dma_start(wo_sb[:], Wo[:])
            bo_rep = big.tile([P, D], f32, name="bo_rep")
            nc.sync.dma_start(bo_rep[:], bo[:].to_broadcast((P, D)))

            # persistent activations: q/k in fp8 DoubleRow slab layout
            kq_sb = [big.tile([P, 2, T], fp8, name=f"kq{r}") for r in range(2)]
            qq_sb = [big.tile([P, 2, T], fp8, name=f"qq{r}") for r in range(2)]
            vp_sb = [big.tile([P, HC, DK + 1], bf16, name=f"vp{t}") for t in range(NKT)]
            for t in range(NKT):
                nc.any.memset(vp_sb[t][:], 1.0)
            ob_sb = [
                [big.tile([P, DC], bf16, name=f"ob{cb}_{qb}") for qb in range(NQB)]
                for cb in range(2)
            ]
            obT_sb = [
                [big.tile([P, QCH], bf16, name=f"obT{cb}_{ds}") for ds in range(4)]
                for cb in range(2)
            ]

            # ---------- projection emitters ----------
            # late_q: (pos, fn) conversions deferred a couple of produce steps
            # so they never park at the head of the in-order Act/DVE queues.
            late_q = []

            def proj_qk(w_sb, bias_sb, dst, dt, c, late_pos=None):
                ps = accp.tile([P, QCH], f32, tag="proj")
                for g in range(NG):
                    nc.tensor.matmul(
                        ps[:],
                        w_sb[:, g, dt, :],
                        xt_sb[g][:, c * QCH : (c + 1) * QCH],
                        start=(g == 0),
                        stop=(g == NG - 1),
                    )

                def finish(ps=ps, dt=dt, c=c):
                    st = stgp.tile([P, QCH], fp8, tag="qkstg")
                    nc.scalar.activation(
                        st[:], ps[:], FT.Identity, bias=bias_sb[:, dt : dt + 1]
                    )
                    r, half = dt // 2, dt % 2
                    nc.sync.dma_start(
                        dst[r][
                            64 * half : 64 * half + 64, :, c * QCH : (c + 1) * QCH
                        ],
                        st[:],
                    )

                if late_pos is None:
                    finish()
                else:
                    late_q.append((late_pos, finish))

            def proj_v(tt, late_pos=None):
                ps = accp.tile([P, QCH], f32, tag="proj")
                for g in range(NG):
                    nc.tensor.matmul(
                        ps[:],
                        xt_sb[g][:, tt * P : (tt + 1) * P],
                        wv_sb[:, g, :],
                        start=(g == 0),
                        stop=(g == NG - 1),
                    )

                def finish(ps=ps, tt=tt):
                    nc.vector.tensor_tensor(
                        vp_sb[tt][:, :, 0:DK],
                        ps[:].rearrange("p (h d) -> p h d", d=DK),
                        bv_rep[:].rearrange("p (h d) -> p h d", d=DK),
                        ADD,
                    )

                if late_pos is None:
                    finish()
                else:
                    late_q.append((late_pos, finish))

            # ---------- attention emitters ----------
            def emit_produce(c, h, g):
                r, j = h // 4, h % 4
                bsl = slice(32 * j, 32 * j + 32)
                qsl = slice(c * QCH, (c + 1) * QCH)
                sg = sgp.tile([P, 2, QCH], f32, tag="sg")
                for i in range(2):
                    kt = 2 * g + i
                    nc.tensor.matmul(
                        sg[:, i, :],
                        kq_sb[r][bsl, :, kt * P : (kt + 1) * P],
                        qq_sb[r][bsl, :, qsl],
                        start=True,
                        stop=True,
                        perf_mode=DR,
                        tile_position=(32 * j, 0),
                    )
                # exp emitted as one op per psum bank so the first bank frees
                # earlier (region-level WAR lets scores(t+2) start sooner)
                pt = ptp.tile([P, 2, QCH], bf16, tag="pt")
                for i in range(2):
                    if g in DVE_GROUPS:
                        nc.vector.tensor_scalar(
                            pt[:, i, :].bitcast(i16),
                            sg[:, i, :],
                            ALPHA,
                            BETA,
                            MUL,
                            ADD,
                        )
                    else:
                        nc.scalar.activation(
                            pt[:, i, :], sg[:, i, :], FT.Exp, scale=0.125
                        )
                return pt

            def emit_consume(c, h, qb, pts):
                cb = c % 2
                av = avp.tile([P, QCH], f32, tag="av")
                for kt in range(NKT):
                    nc.tensor.matmul(
                        av[:, 0:65],
                        pts[kt // 2][:, kt % 2, qb * P : (qb + 1) * P],
                        vp_sb[kt][:, h, :],
                        start=(kt == 0),
                        stop=(kt == NKT - 1),
                    )
                rec = rcp.tile([P, 1], f32, tag="rec")
                nc.vector.reciprocal(rec[:], av[:, 64:65])
                nc.vector.tensor_tensor(
                    ob_sb[cb][qb][:, h * DK : (h + 1) * DK],
                    av[:, 0:DK],
                    rec[:].to_broadcast((P, DK)),
                    MUL,
                )

            def emit_transpose(c, ds):
                cb = c % 2
                trf = avp.tile([P, QCH], f32, tag="av")
                tr = trf[:].bitcast(bf16)
                for qb in range(NQB):
                    nc.tensor.transpose(
                        tr[:, qb * P : (qb + 1) * P],
                        ob_sb[cb][qb][:, ds * P : (ds + 1) * P],
                        iden_sb[:],
                    )
                nc.scalar.activation(obT_sb[cb][ds][:], tr[:, 0:QCH], FT.Copy)

            def emit_oproj(c, mb, tg):
                cb = c % 2
                msl = slice(mb * QCH, (mb + 1) * QCH)
                res = resp.tile([P, 2, QCH], f16, tag="ores")
                for k2 in range(2):
                    qb = 2 * tg + k2
                    ps = accp.tile([P, QCH], f32, tag="proj")
                    for ds in range(4):
                        nc.tensor.matmul(
                            ps[:],
                            obT_sb[cb][ds][:, qb * P : (qb + 1) * P],
                            wo_sb[:, ds, msl],
                            start=(ds == 0),
                            stop=(ds == 3),
                        )
                    nc.vector.tensor_tensor(res[:, k2, :], ps[:], bo_rep[:, msl], ADD)
                ov = out[:].rearrange("(a k p) m -> a p k m", k=2, p=P)
                nc.gpsimd.dma_start(ov[2 * c + tg, :, :, msl], res[:])

            # ---------- preamble: minimal work to start attention ----------
            for c in range(NQC):
                proj_qk(wk_sb, bk_sb, kq_sb, 0, c)
            proj_qk(wq_sb, bq_sb, qq_sb, 0, 0)

            # fillers injected into the produce stream: step -> [closures]
            fillers = {}

            def add_filler(pos, fn):
                fillers.setdefault(pos, []).append(fn)

            for g2 in range(NGR):
                add_filler(g2, lambda tt=2 * g2, p=g2: proj_v(tt, late_pos=p + 2))
                add_filler(g2, lambda tt=2 * g2 + 1, p=g2: proj_v(tt, late_pos=p + 2))
            for dt in range(1, NPAIR):
                base = 8 * dt
                add_filler(
                    base,
                    lambda d=dt, p=base: proj_qk(wk_sb, bk_sb, kq_sb, d, 0, p + 2),
                )
                add_filler(
                    base + 1,
                    lambda d=dt, p=base: proj_qk(wq_sb, bq_sb, qq_sb, d, 0, p + 3),
                )
                for c in range(1, NQC):
                    add_filler(
                        base + 1 + c,
                        lambda d=dt, cc=c, p=base + 1 + c: proj_qk(
                            wk_sb, bk_sb, kq_sb, d, cc, p + 2
                        ),
                    )
            for c in range(1, NQC):
                for dt in range(NPAIR):
                    add_filler(
                        64 * (c - 1) + 32 + 4 * dt,
                        lambda d=dt, cc=c, p=64 * (c - 1) + 32 + 4 * dt: proj_qk(
                            wq_sb, bq_sb, qq_sb, d, cc, p + 2
                        ),
                    )

            # ---------- main interleaved stream ----------
            stream = [
                (c, h, g) for c in range(NQC) for h in range(HC) for g in range(NGR)
            ]
            head_pts = {}
            consume_q = []   # (ready_pos, c, h, qb, pts)
            out_q = []       # (ready_pos, closure)
            t = 0

            def drain(pos, budget_consume=1, budget_out=1):
                nonlocal consume_q, out_q
                done_heads = []
                while consume_q and consume_q[0][0] <= pos and budget_consume > 0:
                    _, cc, hh, qb, pts = consume_q.pop(0)
                    emit_consume(cc, hh, qb, pts)
                    budget_consume -= 1
                    if qb == NQB - 1:
                        done_heads.append((cc, hh))
                while out_q and out_q[0][0] <= pos and budget_out > 0:
                    _, fn = out_q.pop(0)
                    fn()
                    budget_out -= 1
                return done_heads

            for t, (c, h, g) in enumerate(stream):
                while late_q and late_q[0][0] <= t:
                    late_q.pop(0)[1]()
                for fn in fillers.pop(t, ()):
                    fn()
                pt = emit_produce(c, h, g)
                head_pts.setdefault((c, h), []).append(pt)
                if g == NGR - 1:
                    pts = head_pts.pop((c, h))
                    for qb in range(NQB):
                        consume_q.append((t + LAGU + qb, c, h, qb, pts))
                done = drain(t)
                for (cc, hh) in done:
                    if hh == HC - 1:
                        base = t + 2
                        for ds in range(4):
                            out_q.append((base + 2 * ds, lambda c2=cc, d=ds: emit_transpose(c2, d)))
                        k = 0
                        for mb in range(2):
                            for tg in range(2):
                                out_q.append(
                                    (base + 8 + 2 * k, lambda c2=cc, m=mb, t2=tg: emit_oproj(c2, m, t2))
                                )
                                k += 1

            # flush the tail
            while late_q:
                late_q.pop(0)[1]()
            pos = len(stream)
            while consume_q or out_q:
                pos += 1
                done = drain(pos, budget_consume=1, budget_out=1)
                for (cc, hh) in done:
                    if hh == HC - 1:
                        for ds in range(4):
                            out_q.append((pos, lambda c2=cc, d=ds: emit_transpose(c2, d)))
                        for mb in range(2):
                            for tg in range(2):
                                out_q.append((pos, lambda c2=cc, m=mb, t2=tg: emit_oproj(c2, m, t2)))

    nc.compile()
    return nc


def _prep_inputs(x, Wq, bq, Wk, bk, Wv, bv, Wo, bo):
    """Shard + lay out inputs for the 8 cores (batch x head-group)."""
    x = np.asarray(x, dtype=np.float32)
    to_bf = lambda a: np.ascontiguousarray(a).astype(ml_dtypes.bfloat16)
    Wq, Wk, Wv, Wo = (np.asarray(w, np.float32) for w in (Wq, Wk, Wv, Wo))
    bq, bk, bv, bo = (np.asarray(v, np.float32) for v in (bq, bk, bv, bo))
    bo_half = np.ascontiguousarray((bo * 0.5).reshape(1, D))
    iden = np.eye(P, dtype=ml_dtypes.bfloat16)
    xTb = [to_bf(x[b].T.reshape(NG, P, T)) for b in range(B)]
    # fp8-slab column permutation: staging partition p holds q-dim col(dt, p)
    m = np.arange(P)
    colperm = (m % 2) * 32 + (m // 2) % 32  # within 64-dim head block
    colidx = np.stack(
        [(2 * dt + (m // 2) // 32) * 64 + colperm for dt in range(NPAIR)]
    )  # [NPAIR, P]
    in_maps = []
    for core in range(NCORES):
        b, hg = core // 2, core % 2
        csl = slice(hg * DC, (hg + 1) * DC)

        def tile_qk(W):
            Wc = W[:, csl]  # [D, DC]
            # [p, g, dt, m] with permuted columns
            Wt = Wc[:, colidx.reshape(-1)].reshape(D, NPAIR, P)
            return to_bf(Wt.reshape(NG, P, NPAIR, P).transpose(1, 0, 2, 3))

        def stripe_bias(bvec):
            bc = bvec[csl]
            return np.ascontiguousarray(bc[colidx].T)  # [P, NPAIR]

        in_maps.append(
            {
                "xT": xTb[b],
                "Wq": tile_qk(Wq),
                "Wk": tile_qk(Wk),
                "Wv": to_bf(Wv[:, csl].reshape(NG, P, DC).transpose(1, 0, 2)),
                "Wo": to_bf(Wo[csl, :].reshape(4, P, D).transpose(1, 0, 2)),
                "bqp": stripe_bias(bq),
                "bkp": stripe_bias(bk),
                "bv": np.ascontiguousarray(bv[csl].reshape(1, DC)),
                "bo": bo_half,
                "iden": iden,
            }
        )
    return in_maps


def kernel(x, Wq, bq, Wk, bk, Wv, bv, Wo, bo):
    if "nc" not in _CACHE:
        _CACHE["nc"] = build_kernel()
    nc = _CACHE["nc"]
    in_maps = _prep_inputs(x, Wq, bq, Wk, bk, Wv, bv, Wo, bo)
    res = run_bass_kernel_spmd(nc, in_maps, list(range(NCORES)))
    out = np.empty((B, T, D), dtype=np.float32)
    for b in range(B):
        out[b] = res.results[2 * b]["out"].astype(np.float32) + res.results[
            2 * b + 1
        ]["out"].astype(np.float32)
    return out

